# revision 3
# baseline (speedup 1.0000x reference)
"""Trainium2 Bass kernel for nn_Conv2d_lsq_int (LSQ int8-style quantized 3x3 conv).

Full-input contract: kernel(**inputs) takes the complete tensors
(x[16,320,64,64], weight[320,320,3,3], bias[320], scalar step sizes) and
returns the full [16,320,64,64] float32 output.

Distribution: data-parallel over the batch dim — 2 images per NeuronCore on
8 cores; weight/bias replicated. The host shards the batch, quantizes +
Winograd-transforms the weight (fp16, shift_scale folded in), computes the
320-element bias requant, and concatenates the per-core outputs.

Algorithm: 1D Winograd F(2,3) along W, direct 3-tap accumulation along H.
  tx0 = d0-d2, tx1 = d1+d2, tx2' = d1-d2 (= -tx2), tx3 = d1-d3
  tw  = (g0, (g0+g1+g2)/2, (g0-g1+g2)/2, g2) * shift_scale   (host, fp16)
  psum_c = sum_{ci,kh} tw_c * tx_c   =>  (M0, M1, -M2, M3) * ss
  y_even = M0+M1+M2,  y_odd = M1-M2-M3  (DVE, fused with the fp32
  magic-number round of y*ss; shift_scale=2^-7 rides in the weights so the
  PSUM accumulators hold y*ss exactly).

All engine traffic off PSUM is fp16 (DVE 2x tensor-tensor / 4x tensor-scalar
modes need packed 2-byte operands; fp16 holds ints to 2048 exactly, enough
for |tx| <= 254 and all post-clamp values). Input quantization rounds on the
fp16 write: v = +-x/sx + 1536 in [1261,1811] where fp16 ulp is 1, so the
ScalarE activation's fp16 store IS round-to-nearest-even. Odd image columns
are stored negated (o = 1536 - x_int), which makes the three difference
transforms (t0, t1, t3) magic-free fp16 tensor-tensors at 2x rate; t2' needs
one scalar_tensor_tensor (e - 3072) + o. Even/odd columns are staged
de-interleaved so every transform operand is stride-1 packed.

The epilogue rounds via a split magic: M1 + 1.5*2^17 (rounds to 1/64, error
<= 2^-7) then the final scalar_tensor_tensor adds the remaining
1.5*2^23 - 1.5*2^17, producing round(y*ss) with at most +-1 deviation on
~3% of non-saturated outputs (well inside the 2e-2 gate; ~81% of outputs
clamp to +-127 anyway). Clamp chain: GpSimd (min, de-magic) -> DVE 4x
(max -127, +bias) -> DVE 4x (final clamp) -> fp16 DMA out; host casts.

cin = 320 = 2.5*128: the 64-wide remainder is duplicated on partitions
64:127; remainder matmuls run concurrently in the two PE row-groups. The
64-wide cout remainder column-packs two row-pairs into the two column halves
of the array.
"""

import contextlib
import ctypes
import sys
import types

import numpy as np

import concourse.bass as bass  # noqa: F401
import concourse.tile as tile
from concourse import bacc, mybir
from concourse.bass_utils import run_bass_kernel_spmd

F32 = mybir.dt.float32
F16 = mybir.dt.float16
OP = mybir.AluOpType
ACTF = mybir.ActivationFunctionType

MAGH = 1536.0          # 1.5 * 2**10 : fp16 write rounds x*r to integer
MAGF = 12582912.0      # 1.5 * 2**23 : fp32 round-to-nearest-even magic
M64 = 196608.0         # 1.5 * 2**17 : partial magic (rounds to 1/64)
MREST = MAGF - M64     # remainder added by the last combine op
QMAX = 127.0

B, CIN, COUT, H, W, K = 16, 320, 320, 64, 64, 3
N_CORES = 8
IMGS_PER_CORE = B // N_CORES
HW = H * W
S = W // 2          # winograd tiles along W
RH = 34             # staged rows per half-image (32 + halo + pad)
CHUNKS = [(0, 128), (128, 128), (256, 64)]  # (start, size) along cin / cout
WCOLS = 4 * K * COUT  # host-prepped weight row: [comp, kh, co]


def _install_axon_ntff_hook():
    """Slim antenv.axon_hooks so trace=True works (and never crashes) here."""
    if "antenv.axon_hooks" in sys.modules:
        return
    hook = None
    try:
        lib = ctypes.CDLL("/opt/axon/libaxon_pjrt.so")
        if hasattr(lib, "axon_start_nrt_profile"):
            lib.axon_start_nrt_profile.argtypes = [
                ctypes.POINTER(ctypes.c_int64),
                ctypes.c_size_t,
            ]
            lib.axon_start_nrt_profile.restype = ctypes.c_int64
            lib.axon_stop_nrt_profile.argtypes = [ctypes.c_char_p]
            lib.axon_stop_nrt_profile.restype = ctypes.c_int64

            @contextlib.contextmanager
            def hook(output_dir, device_ids):  # noqa: F811
                import jax

                jax.devices()
                if device_ids:
                    ids = (ctypes.c_int64 * len(device_ids))(*device_ids)
                    rc = lib.axon_start_nrt_profile(ids, len(device_ids))
                else:
                    rc = lib.axon_start_nrt_profile(None, 0)
                if rc != 0:
                    raise RuntimeError(f"axon_start_nrt_profile rc={rc}")
                try:
                    yield
                finally:
                    n = lib.axon_stop_nrt_profile(str(output_dir).encode())
                    print(f"profile: {n} ntff file(s) -> {output_dir}",
                          file=sys.stderr)
    except OSError:
        pass

    mod = types.ModuleType("antenv.axon_hooks")
    mod.get_axon_ntff_profile_hook = lambda: hook
    mod.set_axon_ntff_profile_hook = lambda h: None
    sys.modules["antenv.axon_hooks"] = mod

    # keep profiling artifacts local (zero-egress container)
    import concourse.bass_utils as bu

    bu.upload_artifacts = lambda tmpdir: "local://" + str(tmpdir)


def bias_int8(b, sb, ss, sx, sw):
    """Host fp32 replica of the reference's bias requant.

    Every op is a single IEEE-754 fp32 operation in the reference's exact
    order, so this is bit-identical to the jax fp32 computation.
    """
    f32 = np.float32
    b = np.asarray(b, np.float32)
    b_deq = np.clip(np.round(b / f32(sb)), -QMAX, QMAX).astype(np.float32) * f32(sb)
    x_scale = f32(1.0) / f32(sx)
    w_scale = f32(1.0) / f32(sw)
    t = ((b_deq * f32(ss)) * x_scale) * w_scale
    return np.clip(np.round(t), -QMAX, QMAX).astype(np.float32)


def prep_weight(w, sw, ss):
    """Host weight prep: quantize + 1D Winograd row transform, fp16.

    [co, ci, kh, kw] -> [ci, (comp kh) co] with comp in
    (g0, (g0+g1+g2)/2, (g0-g1+g2)/2, g2), all scaled by shift_scale.
    w_int is integer in [-127,127]; comp values are k/2 * 2^-7 with
    |k| <= 381 -- exact in fp16.
    """
    f32 = np.float32
    w = np.asarray(w, np.float32)
    w_int = np.clip(np.round(w / f32(sw)), -QMAX, QMAX).astype(np.float32)
    g0 = w_int[:, :, :, 0]
    g1 = w_int[:, :, :, 1]
    g2 = w_int[:, :, :, 2]
    comps = np.stack(
        [g0, (g0 + g1 + g2) * 0.5, (g0 - g1 + g2) * 0.5, g2], axis=0
    ) * f32(ss)  # [comp, co, ci, kh]
    out = np.transpose(comps, (2, 0, 3, 1))  # [ci, comp, kh, co]
    return np.ascontiguousarray(out).reshape(CIN, WCOLS).astype(np.float16)


def _build(sx: float):
    """Build the per-core Bass program. step_x is baked as an immediate."""
    nc = bacc.Bacc("TRN2", target_bir_lowering=False, debug=False)

    x_d = nc.dram_tensor("x", [IMGS_PER_CORE, CIN, HW], F32, kind="ExternalInput")
    w_d = nc.dram_tensor("w", [CIN, WCOLS], F16, kind="ExternalInput")
    b_d = nc.dram_tensor("b", [COUT], F32, kind="ExternalInput")
    y_d = nc.dram_tensor("y", [IMGS_PER_CORE, COUT, HW], F16, kind="ExternalOutput")

    r_x = float(np.float32(1.0) / np.float32(sx))  # x_scale

    with tile.TileContext(nc) as tc:
        with (
            tc.tile_pool(name="persist", bufs=1) as persist,
            tc.tile_pool(name="raw", bufs=3) as rawp,
            tc.tile_pool(name="epi", bufs=3) as epi,
            tc.tile_pool(name="psum", bufs=8, space="PSUM") as psum,
        ):
            # ---------- persistent staging + transform buffers --------------
            # st[(c,h)]: fp16 quantized staging, [p, parity, RH, 32] with
            # even cols as +x_int+MAGH, odd cols as -x_int+MAGH (negated).
            # tx[(c,h)]: fp16 winograd row-transform, [128, 4 x RH x S]
            st = {}
            txt = {}
            for c in range(len(CHUNKS)):
                for h in range(2):
                    st[(c, h)] = persist.tile(
                        [128, 2 * RH * S], F16, tag=f"st{c}_{h}",
                        name=f"st{c}_{h}",
                    )
                    txt[(c, h)] = persist.tile(
                        [128, 4 * RH * S], F16, tag=f"tx{c}_{h}",
                        name=f"tx{c}_{h}",
                    )
                    s4 = st[(c, h)].rearrange(
                        "p (par r s) -> p par r s", par=2, r=RH
                    )
                    # pad row (top for h=0, bottom for h=1) = MAGH ("0")
                    prow = 0 if h == 0 else RH - 1
                    nc.vector.memset(s4[:, :, prow : prow + 1, :], MAGH)
                    if c == 0 and h == 0:
                        # value-preserving dummy: pulls the lazy
                        # ACT_TABLE_LOAD off the quant critical path
                        nc.scalar.activation(
                            s4[:, 0, prow : prow + 1, :],
                            s4[:, 0, prow : prow + 1, :], ACTF.Copy,
                        )

            # ---------------- weights: host-prepped fp16 DMA ----------------
            wq = {}
            for c, (ci0, pc) in enumerate(CHUNKS):
                wq[c] = persist.tile(
                    [128, WCOLS], F16, tag=f"wq{c}", name=f"wq{c}"
                )
                qtr = WCOLS // 4
                for lo in range(0, WCOLS, qtr):
                    hi = lo + qtr
                    nc.sync.dma_start(
                        wq[c][:pc, lo:hi], w_d[ci0 : ci0 + pc, lo:hi]
                    )
                    if pc < 128:
                        nc.sync.dma_start(
                            wq[c][pc : 2 * pc, lo:hi],
                            w_d[ci0 : ci0 + pc, lo:hi],
                        )

            # ------------- x: DMA + quantize (fp16 magic) + transform -------
            def emit_x_half(i, h, only_c=None):
                # image rows covered: 32h-1 .. 32h+32 (halo refetch), the
                # missing edge row is the persistent MAGH pad row
                r_img0 = 32 * h - 1
                dst_r0 = 0
                if h == 0:
                    r_img0, dst_r0 = 0, 1
                for c, (ci0, pc) in enumerate(CHUNKS):
                    if only_c is not None and c != only_c:
                        continue
                    raw = rawp.tile([128, 33 * W], F32, tag="raw",
                                    name=f"raw{i}_{h}_{c}")
                    r3 = raw.rearrange("p (r w) -> p r w", r=33)
                    qp = pc if pc == 128 else 2 * pc
                    for a, b in ((0, 9), (9, 17), (17, 25), (25, 33)):
                        srcp = x_d[
                            i, ci0 : ci0 + pc,
                            (r_img0 + a) * W : (r_img0 + b) * W
                        ].rearrange("p (r w) -> p r w", r=b - a)
                        nc.sync.dma_start(r3[:pc, a:b, :], srcp)
                        if pc < 128:
                            nc.sync.dma_start(r3[pc : 2 * pc, a:b, :], srcp)
                    # quantize: fp16 store rounds; odd cols negated
                    r4 = raw.rearrange("p (r s two) -> p r s two", r=33,
                                       two=2)
                    s4 = st[(c, h)].rearrange(
                        "p (par r s) -> p par r s", par=2, r=RH
                    )
                    ev = s4[:qp, 0, dst_r0 : dst_r0 + 33, :]
                    od = s4[:qp, 1, dst_r0 : dst_r0 + 33, :]
                    nc.scalar.activation(
                        ev, r4[:qp, :, :, 0:1].squeeze(3), ACTF.Copy,
                        bias=MAGH, scale=r_x,
                    )
                    nc.scalar.activation(
                        od, r4[:qp, :, :, 1:2].squeeze(3), ACTF.Copy,
                        bias=MAGH, scale=-r_x,
                    )
                    both = s4[:qp, :, dst_r0 : dst_r0 + 33, :]
                    nc.vector.tensor_scalar(
                        both, both, MAGH + QMAX, MAGH - QMAX, OP.min, OP.max
                    )
                    # winograd row transform (magic-free differences):
                    # t0[s] = o[s]-o[s-1], t1 = e-o, t2' = (e-2*MAGH)+o,
                    # t3[s] = e[s]-e[s+1]; edges subtract MAGH (pad col = 0)
                    e = s4[:qp, 0]
                    o = s4[:qp, 1]
                    tv = txt[(c, h)].rearrange("p (c r s) -> p c r s", c=4,
                                               r=RH)
                    t0 = tv[:qp, 0:1].squeeze(1)
                    t1 = tv[:qp, 1:2].squeeze(1)
                    t2 = tv[:qp, 2:3].squeeze(1)
                    t3 = tv[:qp, 3:4].squeeze(1)
                    nc.vector.tensor_tensor(
                        t0[:, :, 1:S], o[:, :, 1:S], o[:, :, 0 : S - 1],
                        OP.subtract,
                    )
                    nc.vector.tensor_scalar(
                        t0[:, :, 0:1], o[:, :, 0:1], MAGH, None, OP.subtract
                    )
                    nc.gpsimd.tensor_tensor(t1, e, o, OP.subtract)
                    nc.vector.scalar_tensor_tensor(
                        t2, e, -2.0 * MAGH, o, OP.add, OP.add
                    )
                    nc.vector.tensor_tensor(
                        t3[:, :, 0 : S - 1], e[:, :, 0 : S - 1],
                        e[:, :, 1:S], OP.subtract,
                    )
                    nc.vector.tensor_scalar(
                        t3[:, :, S - 1 : S], e[:, :, S - 1 : S], MAGH, None,
                        OP.subtract,
                    )

            # emission order tuned so the first matmul group's deps land
            # early; h1 and image-1 staging are prefetched inside the loop
            emit_x_half(0, 0, only_c=0)
            emit_x_half(0, 0, only_c=1)
            emit_x_half(0, 0, only_c=2)

            # ------------- b_int8 (host-computed), laid out [128, 3] --------
            # col 2 holds the cout remainder on both partition halves
            bt = persist.tile([128, 3], F32, tag="bias", name="bias")
            nc.sync.dma_start(
                bt[:, 0:2], b_d[0:256].rearrange("(c p) -> p c", p=128)
            )
            nc.sync.dma_start(
                bt[:64, 2:3], b_d[256:320].rearrange("(p c) -> p c", c=1)
            )
            nc.sync.dma_start(
                bt[64:128, 2:3], b_d[256:320].rearrange("(p c) -> p c", c=1)
            )

            # ---------------- main conv loop --------------------------------
            def wslice(comp, kh, q, co0, cs, lo=0, hi=128):
                base = (comp * K + kh) * COUT
                return wq[q][lo:hi, base + co0 : base + co0 + cs]

            def rhs(q, h, comp, r0, nr, lo=0, hi=128):
                tv = txt[(q, h)].rearrange("p (c r s) -> p c r s", c=4, r=RH)
                return tv[lo:hi, comp : comp + 1, r0 : r0 + nr, :].squeeze(1)

            def emit_outtf_epi(ps, i, cot, co0, cs, rows, yparts):
                # ps: 4 psum tiles [P, Wp] holding (M0, M1, -M2, M3)*ss
                P = 128 if cs < 128 else cs
                Wp = ps[0].shape[1]
                # c1m = M1 + M64 (partial magic round to 1/64)
                c1m = epi.tile([128, 512], F32, tag="c1", name="c1")
                te = epi.tile([128, 512], F32, tag="te", name="te")
                to = epi.tile([128, 512], F32, tag="to", name="to")
                nc.scalar.activation(c1m[:P, :Wp], ps[1][:P], ACTF.Copy,
                                     bias=M64)
                nc.vector.tensor_tensor(te[:P, :Wp], c1m[:P, :Wp], ps[0][:P],
                                        OP.add)
                nc.vector.tensor_tensor(to[:P, :Wp], c1m[:P, :Wp], ps[2][:P],
                                        OP.add)
                yi = epi.tile([128, 1024], F32, tag="yi", name="yi")
                wid = 2 * Wp
                # even = (te + MREST) - (-M2); odd = (to + MREST) - M3;
                # the full-magic add rounds y*ss to integer
                nc.vector.scalar_tensor_tensor(
                    yi[:P, 0:Wp], te[:P, :Wp], MREST, ps[2][:P],
                    OP.add, OP.subtract,
                )
                nc.vector.scalar_tensor_tensor(
                    yi[:P, Wp:wid], to[:P, :Wp], MREST, ps[3][:P],
                    OP.add, OP.subtract,
                )
                # clamp+bias chain: (min, de-magic) -> (max, +b) -> clamp
                w16 = epi.tile([128, 1024], F16, tag="w16", name="w16")
                o16 = epi.tile([128, 1024], F16, tag="o16", name="o16")
                nc.gpsimd.tensor_scalar(
                    w16[:P, :wid], yi[:P, :wid], MAGF + QMAX, MAGF,
                    OP.min, OP.subtract,
                )
                nc.vector.tensor_scalar(
                    w16[:P, :wid], w16[:P, :wid], -QMAX,
                    bt[:P, cot : cot + 1], OP.max, OP.add,
                )
                nc.vector.tensor_scalar(
                    o16[:P, :wid], w16[:P, :wid], QMAX, -QMAX,
                    OP.min, OP.max,
                )
                # output dram layout is [i, co, eo, r, s]; host de-interleaves
                nps = rows * S
                for part_lo, r0 in yparts:
                    for eo in range(2):
                        nc.sync.dma_start(
                            y_d[i, co0 : co0 + cs,
                                eo * (HW // 2) + r0 * S :
                                eo * (HW // 2) + r0 * S + nps],
                            o16[part_lo : part_lo + cs,
                                eo * Wp : eo * Wp + nps],
                        )

            def emit_group_full(i, p, cot):
                # cout chunks 0/1 (cs=128): one psum bank per comp, free 512
                h, po = divmod(p, 2)
                co0, cs = CHUNKS[cot]
                ps = [psum.tile([128, 512], F32, tag="ps", name=f"ps{_c}")
                      for _c in range(4)]
                for q in (0, 1):
                    for comp in (0, 3, 1, 2):
                        for kh in range(K):
                            nc.tensor.matmul(
                                ps[comp][:cs, :],
                                wslice(comp, kh, q, co0, cs),
                                rhs(q, h, comp, 16 * po + kh, 16),
                                start=(q == 0 and kh == 0),
                                stop=False,
                            )
                # cin remainder: pack comp pairs into the two PE row groups
                # (the 64 cin channels are duplicated on partitions 64:127)
                for kh in range(K):
                    for comp in range(4):
                        lo = 0 if comp % 2 == 0 else 64
                        nc.tensor.matmul(
                            ps[comp][:cs, :],
                            wslice(comp, kh, 2, co0, cs, lo, lo + 64),
                            rhs(2, h, comp, 16 * po + kh, 16, lo, lo + 64),
                            start=False,
                            stop=(kh == 2),
                        )
                emit_outtf_epi(ps, i, cot, co0, cs, 16, [(0, 16 * p)])

            def emit_group_rem(i, pe):
                # column-pack row-pairs pe, pe+1 into the two column halves
                co0, cs = CHUNKS[2]
                h = pe // 2
                poA, poB = pe % 2, (pe + 1) % 2
                ps = [psum.tile([128, 512], F32, tag="ps", name=f"psr{_c}")
                      for _c in range(4)]
                for q in (0, 1):
                    for comp in range(4):
                        for kh in range(K):
                            first = q == 0 and kh == 0
                            w_ = wslice(comp, kh, q, co0, cs)
                            nc.tensor.matmul(
                                ps[comp][0:cs, :], w_,
                                rhs(q, h, comp, 16 * poA + kh, 16),
                                start=first, stop=False,
                                tile_position=(0, 0),
                            )
                            nc.tensor.matmul(
                                ps[comp][64 : 64 + cs, :], w_,
                                rhs(q, h, comp, 16 * poB + kh, 16),
                                start=first, stop=False,
                                tile_position=(0, 64),
                            )
                for comp in range(4):
                    for kh in range(K):
                        last = kh == 2
                        nc.tensor.matmul(
                            ps[comp][0:cs, :],
                            wslice(comp, kh, 2, co0, cs, 0, 64),
                            rhs(2, h, comp, 16 * poA + kh, 16, 0, 64),
                            start=False, stop=last,
                            tile_position=(0, 0),
                        )
                        nc.tensor.matmul(
                            ps[comp][64 : 64 + cs, :],
                            wslice(comp, kh, 2, co0, cs, 64, 128),
                            rhs(2, h, comp, 16 * poB + kh, 16, 64, 128),
                            start=False, stop=last,
                            tile_position=(64, 64),
                        )
                emit_outtf_epi(ps, i, 2, co0, cs, 16,
                               [(0, 16 * pe), (64, 16 * (pe + 1))])

            for i in range(IMGS_PER_CORE):
                for p in range(4):
                    emit_group_full(i, p, 0)
                    # (0,1) staging is a first write (no WAR hazard): spread
                    # its chunks between groups so the DVE bursts stay short
                    if i == 0 and p == 0:
                        emit_x_half(0, 1, only_c=0)
                    emit_group_full(i, p, 1)
                    if i == 0 and p == 0:
                        emit_x_half(0, 1, only_c=1)
                    if p % 2 == 1:
                        emit_group_rem(i, p - 1)
                    if i == 0 and p == 0:
                        emit_x_half(0, 1, only_c=2)
                    # image i+1 staging overwrites tiles read by pairs
                    # 2h, 2h+1 incl. the rem group -> emit only after it
                    if i + 1 < IMGS_PER_CORE and p in (1, 3):
                        for c_ in range(3):
                            emit_x_half(i + 1, p // 2, only_c=c_)

    nc.compile()
    return nc


_BUILD_CACHE = {}


def _get_nc(sx):
    if sx not in _BUILD_CACHE:
        _BUILD_CACHE[sx] = _build(sx)
    return _BUILD_CACHE[sx]


def _run(x, weight, bias, step_x, step_w, step_b, shift_scale, trace=False):
    _install_axon_ntff_hook()
    x = np.ascontiguousarray(np.asarray(x, dtype=np.float32))
    w = np.asarray(weight, dtype=np.float32)
    b = np.ascontiguousarray(np.asarray(bias, dtype=np.float32))
    sx = float(np.asarray(step_x))
    sw = float(np.asarray(step_w))
    sb = float(np.asarray(step_b))
    ss = float(np.asarray(shift_scale))

    nc = _get_nc(sx)

    w_t = prep_weight(w, sw, ss)
    x_sh = x.reshape(N_CORES, IMGS_PER_CORE, CIN, HW)

    b_i8 = bias_int8(b, sb, ss, sx, sw)
    in_maps = [
        {"x": x_sh[core], "w": w_t, "b": b_i8} for core in range(N_CORES)
    ]
    res = run_bass_kernel_spmd(
        nc, in_maps, core_ids=list(range(N_CORES)), trace=trace
    )
    # device wrote [i, co, eo, r, s] fp16; de-interleave eo into the W axis
    out = np.concatenate(
        [res.results[core]["y"].reshape(IMGS_PER_CORE, COUT, 2, H, S)
         for core in range(N_CORES)],
        axis=0,
    )
    out = np.ascontiguousarray(
        np.transpose(out, (0, 1, 3, 4, 2)).astype(np.float32)
    ).reshape(B, COUT, H, W)
    return out, res


def kernel(x, weight, bias, step_x, step_w, step_b, shift_scale):
    out, _ = _run(x, weight, bias, step_x, step_w, step_b, shift_scale)
    return out


def kernel_profiled(x, weight, bias, step_x, step_w, step_b, shift_scale):
    return _run(x, weight, bias, step_x, step_w, step_b, shift_scale, trace=True)


# revision 13
# speedup vs baseline: 2.3140x; 2.3140x over previous
"""Trainium2 Bass kernel for nn_Conv2d_lsq_int (LSQ int8-style quantized 3x3 conv).

Full-input contract: kernel(**inputs) takes the complete tensors
(x[16,320,64,64], weight[320,320,3,3], bias[320], scalar step sizes) and
returns the full [16,320,64,64] float32 output.

Distribution: data-parallel over the batch dim — 2 images per NeuronCore on
8 cores; weight/bias replicated. The host shards the batch, quantizes +
Winograd-transforms the weight (fp16, shift_scale folded in), computes the
320-element bias requant, and concatenates the per-core outputs.

Algorithm: 1D Winograd F(2,3) along W, direct 3-tap accumulation along H.
  tx0 = d0-d2, tx1 = d1+d2, tx2' = d1-d2 (= -tx2), tx3 = d1-d3
  tw  = (g0, (g0+g1+g2)/2, (g0-g1+g2)/2, g2) * shift_scale   (host, fp16)
  psum_c = sum_{ci,kh} tw_c * tx_c   =>  (M0, M1, -M2, M3) * ss
  y_even = M0+M1+M2,  y_odd = M1-M2-M3  (DVE, fused with the fp32
  magic-number round of y*ss; shift_scale=2^-7 rides in the weights so the
  PSUM accumulators hold y*ss exactly).

All engine traffic off PSUM is fp16 (DVE 2x tensor-tensor / 4x tensor-scalar
modes need packed 2-byte operands; fp16 holds ints to 2048 exactly, enough
for |tx| <= 254 and all post-clamp values). Input quantization rounds on the
fp16 write: v = +-x/sx + 1536 in [1261,1811] where fp16 ulp is 1, so the
ScalarE activation's fp16 store IS round-to-nearest-even. Odd image columns
are stored negated (o = 1536 - x_int), which makes the three difference
transforms (t0, t1, t3) magic-free fp16 tensor-tensors at 2x rate; t2' needs
one scalar_tensor_tensor (e - 3072) + o. Even/odd columns are staged
de-interleaved so every transform operand is stride-1 packed.

The epilogue rounds via a split magic: M1 + 1.5*2^17 (rounds to 1/64, error
<= 2^-7) then the final scalar_tensor_tensor adds the remaining
1.5*2^23 - 1.5*2^17, producing round(y*ss) with at most +-1 deviation on
~3% of non-saturated outputs (well inside the 2e-2 gate; ~81% of outputs
clamp to +-127 anyway). Clamp chain: DVE 2x fp32 clamp in magic space ->
ScalarE Identity(+ b_int8 - MAGF per-partition bias: de-magic and bias in
one pass, fp16 out) -> DVE 4x final clamp -> fp16 DMA out; host casts.

cin = 320 = 2.5*128: the 64-wide remainder is duplicated on partitions
64:127; remainder matmuls run concurrently in the two PE row-groups. The
64-wide cout remainder column-packs two row-pairs into the two column halves
of the array.
"""

import contextlib
import ctypes
import sys
import types

import numpy as np

import concourse.bass as bass  # noqa: F401
import concourse.tile as tile
from concourse import bacc, mybir
from concourse.bass_utils import run_bass_kernel_spmd

F32 = mybir.dt.float32
F16 = mybir.dt.float16
OP = mybir.AluOpType
ACTF = mybir.ActivationFunctionType

MAGH = 1536.0          # 1.5 * 2**10 : fp16 write rounds x*r to integer
MAGF = 12582912.0      # 1.5 * 2**23 : fp32 round-to-nearest-even magic
M64 = 196608.0         # 1.5 * 2**17 : partial magic (rounds to 1/64)
MREST = MAGF - M64     # remainder added by the last combine op
QMAX = 127.0

B, CIN, COUT, H, W, K = 16, 320, 320, 64, 64, 3
N_CORES = 8
IMGS_PER_CORE = B // N_CORES
HW = H * W
S = W // 2          # winograd tiles along W
RH = 34             # staged rows per half-image (32 + halo + pad)
CHUNKS = [(0, 128), (128, 128), (256, 64)]  # (start, size) along cin / cout
WCOLS = 4 * K * COUT  # host-prepped weight row: [comp, kh, co]


def _install_axon_ntff_hook():
    """Slim antenv.axon_hooks so trace=True works (and never crashes) here."""
    if "antenv.axon_hooks" in sys.modules:
        return
    hook = None
    try:
        lib = ctypes.CDLL("/opt/axon/libaxon_pjrt.so")
        if hasattr(lib, "axon_start_nrt_profile"):
            lib.axon_start_nrt_profile.argtypes = [
                ctypes.POINTER(ctypes.c_int64),
                ctypes.c_size_t,
            ]
            lib.axon_start_nrt_profile.restype = ctypes.c_int64
            lib.axon_stop_nrt_profile.argtypes = [ctypes.c_char_p]
            lib.axon_stop_nrt_profile.restype = ctypes.c_int64

            @contextlib.contextmanager
            def hook(output_dir, device_ids):  # noqa: F811
                import jax

                jax.devices()
                if device_ids:
                    ids = (ctypes.c_int64 * len(device_ids))(*device_ids)
                    rc = lib.axon_start_nrt_profile(ids, len(device_ids))
                else:
                    rc = lib.axon_start_nrt_profile(None, 0)
                if rc != 0:
                    raise RuntimeError(f"axon_start_nrt_profile rc={rc}")
                try:
                    yield
                finally:
                    n = lib.axon_stop_nrt_profile(str(output_dir).encode())
                    print(f"profile: {n} ntff file(s) -> {output_dir}",
                          file=sys.stderr)
    except OSError:
        pass

    mod = types.ModuleType("antenv.axon_hooks")
    mod.get_axon_ntff_profile_hook = lambda: hook
    mod.set_axon_ntff_profile_hook = lambda h: None
    sys.modules["antenv.axon_hooks"] = mod

    # keep profiling artifacts local (zero-egress container)
    import concourse.bass_utils as bu

    bu.upload_artifacts = lambda tmpdir: "local://" + str(tmpdir)


def bias_int8(b, sb, ss, sx, sw):
    """Host fp32 replica of the reference's bias requant.

    Every op is a single IEEE-754 fp32 operation in the reference's exact
    order, so this is bit-identical to the jax fp32 computation.
    """
    f32 = np.float32
    b = np.asarray(b, np.float32)
    b_deq = np.clip(np.round(b / f32(sb)), -QMAX, QMAX).astype(np.float32) * f32(sb)
    x_scale = f32(1.0) / f32(sx)
    w_scale = f32(1.0) / f32(sw)
    t = ((b_deq * f32(ss)) * x_scale) * w_scale
    return np.clip(np.round(t), -QMAX, QMAX).astype(np.float32)


def prep_weight(w, sw, ss):
    """Host weight prep: quantize + 1D Winograd row transform, fp16.

    [co, ci, kh, kw] -> [ci, (comp kh) co] with comp in
    (g0, (g0+g1+g2)/2, (g0-g1+g2)/2, g2), all scaled by shift_scale.
    w_int is integer in [-127,127]; comp values are k/2 * 2^-7 with
    |k| <= 381 -- exact in fp16.
    """
    f32 = np.float32
    w = np.asarray(w, np.float32)
    w_int = np.clip(np.round(w / f32(sw)), -QMAX, QMAX).astype(np.float32)
    g0 = w_int[:, :, :, 0]
    g1 = w_int[:, :, :, 1]
    g2 = w_int[:, :, :, 2]
    comps = np.stack(
        [g0, (g0 + g1 + g2) * 0.5, (g0 - g1 + g2) * 0.5, g2], axis=0
    ) * f32(ss)  # [comp, co, ci, kh]
    out = np.transpose(comps, (2, 0, 3, 1))  # [ci, comp, kh, co]
    return np.ascontiguousarray(out).reshape(CIN, WCOLS).astype(np.float16)


def _build(sx: float):
    """Build the per-core Bass program. step_x is baked as an immediate."""
    nc = bacc.Bacc("TRN2", target_bir_lowering=False, debug=False)

    x_d = nc.dram_tensor("x", [IMGS_PER_CORE, CIN, HW], F32, kind="ExternalInput")
    w_d = nc.dram_tensor("w", [CIN, WCOLS], F16, kind="ExternalInput")
    b_d = nc.dram_tensor("b", [COUT], F32, kind="ExternalInput")
    y_d = nc.dram_tensor("y", [IMGS_PER_CORE, COUT, HW], F16, kind="ExternalOutput")

    r_x = float(np.float32(1.0) / np.float32(sx))  # x_scale

    with tile.TileContext(nc) as tc:
        with (
            tc.tile_pool(name="persist", bufs=1) as persist,
            tc.tile_pool(name="raw", bufs=3) as rawp,
            tc.tile_pool(name="epi", bufs=3) as epi,
            tc.tile_pool(name="psum", bufs=8, space="PSUM") as psum,
        ):
            # ---------- persistent staging + transform buffers --------------
            # st[(c,h)]: fp16 quantized staging, [p, parity, RH, 32] with
            # even cols as +x_int+MAGH, odd cols as -x_int+MAGH (negated).
            # tx[(c,h)]: fp16 winograd row-transform, [128, 4 x RH x S]
            # st[(c,h)]: [p, parity, RH, 33]; even block holds data in cols
            # 0..31 with pad col 32, odd block in cols 1..32 with pad col 0,
            # so every winograd difference (incl. the W edges) is one
            # full-width packed op. Pad rows + pad cols stay MAGH forever.
            SW1 = S + 1
            st = {}
            txt = {}
            for c in range(len(CHUNKS)):
                for h in range(2):
                    st[(c, h)] = persist.tile(
                        [128, 2 * RH * SW1], F16, tag=f"st{c}_{h}",
                        name=f"st{c}_{h}",
                    )
                    txt[(c, h)] = persist.tile(
                        [128, 4 * RH * S], F16, tag=f"tx{c}_{h}",
                        name=f"tx{c}_{h}",
                    )
                    nc.vector.memset(st[(c, h)], MAGH)
                    if c == 0 and h == 0:
                        # value-preserving dummy: pulls the lazy
                        # ACT_TABLE_LOAD off the quant critical path
                        s5 = st[(c, h)].rearrange(
                            "p (par r s) -> p par r s", par=2, r=RH
                        )
                        nc.scalar.activation(
                            s5[:, 0, 0:1, :], s5[:, 0, 0:1, :], ACTF.Copy
                        )

            # ---------------- weights: host-prepped fp16 DMA ----------------
            wq = {}
            for c, (ci0, pc) in enumerate(CHUNKS):
                wq[c] = persist.tile(
                    [128, WCOLS], F16, tag=f"wq{c}", name=f"wq{c}"
                )
                qtr = WCOLS // 4
                for lo in range(0, WCOLS, qtr):
                    hi = lo + qtr
                    nc.sync.dma_start(
                        wq[c][:pc, lo:hi], w_d[ci0 : ci0 + pc, lo:hi]
                    )
                    if pc < 128:
                        nc.sync.dma_start(
                            wq[c][pc : 2 * pc, lo:hi],
                            w_d[ci0 : ci0 + pc, lo:hi],
                        )

            # ------------- x: DMA + quantize (fp16 magic) + transform -------
            def emit_x_half(i, h, only_c=None):
                # image rows covered: 32h-1 .. 32h+32 (halo refetch), the
                # missing edge row is the persistent MAGH pad row
                r_img0 = 32 * h - 1
                dst_r0 = 0
                if h == 0:
                    r_img0, dst_r0 = 0, 1
                for c, (ci0, pc) in enumerate(CHUNKS):
                    if only_c is not None and c != only_c:
                        continue
                    raw = rawp.tile([128, 33 * W], F32, tag="raw",
                                    name=f"raw{i}_{h}_{c}")
                    r3 = raw.rearrange("p (r w) -> p r w", r=33)
                    qp = pc if pc == 128 else 2 * pc
                    for a, b in ((0, 9), (9, 17), (17, 25), (25, 33)):
                        srcp = x_d[
                            i, ci0 : ci0 + pc,
                            (r_img0 + a) * W : (r_img0 + b) * W
                        ].rearrange("p (r w) -> p r w", r=b - a)
                        nc.sync.dma_start(r3[:pc, a:b, :], srcp)
                        if pc < 128:
                            nc.sync.dma_start(r3[pc : 2 * pc, a:b, :], srcp)
                    # quantize: fp16 store rounds; odd cols negated
                    r4 = raw.rearrange("p (r s two) -> p r s two", r=33,
                                       two=2)
                    s5 = st[(c, h)].rearrange(
                        "p (par r s) -> p par r s", par=2, r=RH, s=SW1
                    )
                    ev = s5[:qp, 0, dst_r0 : dst_r0 + 33, 0:S]
                    od = s5[:qp, 1, dst_r0 : dst_r0 + 33, 1:SW1]
                    nc.scalar.activation(
                        ev, r4[:qp, :, :, 0:1].squeeze(3), ACTF.Copy,
                        bias=MAGH, scale=r_x,
                    )
                    nc.scalar.activation(
                        od, r4[:qp, :, :, 1:2].squeeze(3), ACTF.Copy,
                        bias=MAGH, scale=-r_x,
                    )
                    # clamp the whole tile flat (pads are MAGH fixpoints)
                    nc.vector.tensor_scalar(
                        st[(c, h)][:qp], st[(c, h)][:qp],
                        MAGH + QMAX, MAGH - QMAX, OP.min, OP.max,
                    )
                    # winograd row transform (magic-free differences, the
                    # pad cols supply the W-edge zeros):
                    # t0 = o[1:]-o[:-1], t1 = e-o, t2' = (e-2*MAGH)+o,
                    # t3 = e[:-1]-e[1:]
                    e0 = s5[:qp, 0, :, 0:S]
                    e1 = s5[:qp, 0, :, 1:SW1]
                    o0 = s5[:qp, 1, :, 0:S]
                    o1 = s5[:qp, 1, :, 1:SW1]
                    tv = txt[(c, h)].rearrange("p (c r s) -> p c r s", c=4,
                                               r=RH)
                    t0 = tv[:qp, 0:1].squeeze(1)
                    t1 = tv[:qp, 1:2].squeeze(1)
                    t2 = tv[:qp, 2:3].squeeze(1)
                    t3 = tv[:qp, 3:4].squeeze(1)
                    nc.vector.tensor_tensor(t0, o1, o0, OP.subtract)
                    nc.gpsimd.tensor_tensor(t1, e0, o1, OP.subtract)
                    nc.vector.scalar_tensor_tensor(
                        t2, e0, -2.0 * MAGH, o1, OP.add, OP.add
                    )
                    nc.vector.tensor_tensor(t3, e0, e1, OP.subtract)

            # emission order tuned so the first matmul group's deps land
            # early; h1 and image-1 staging are prefetched inside the loop
            emit_x_half(0, 0, only_c=0)
            emit_x_half(0, 0, only_c=1)
            emit_x_half(0, 0, only_c=2)

            # ------------- b_int8 (host-computed), laid out [128, 3] --------
            # col 2 holds the cout remainder on both partition halves
            bt = persist.tile([128, 3], F32, tag="bias", name="bias")
            nc.sync.dma_start(
                bt[:, 0:2], b_d[0:256].rearrange("(c p) -> p c", p=128)
            )
            nc.sync.dma_start(
                bt[:64, 2:3], b_d[256:320].rearrange("(p c) -> p c", c=1)
            )
            nc.sync.dma_start(
                bt[64:128, 2:3], b_d[256:320].rearrange("(p c) -> p c", c=1)
            )

            # ---------------- main conv loop --------------------------------
            def wslice(comp, kh, q, co0, cs, lo=0, hi=128):
                base = (comp * K + kh) * COUT
                return wq[q][lo:hi, base + co0 : base + co0 + cs]

            def rhs(q, h, comp, r0, nr, lo=0, hi=128):
                tv = txt[(q, h)].rearrange("p (c r s) -> p c r s", c=4, r=RH)
                return tv[lo:hi, comp : comp + 1, r0 : r0 + nr, :].squeeze(1)

            def emit_outtf_epi(ps, i, cot, co0, cs, rows, yparts):
                # ps: 4 psum tiles [P, Wp] holding (M0, M1, -M2, M3)*ss
                P = 128 if cs < 128 else cs
                Wp = ps[0].shape[1]
                # c1m = M1 + M64 (partial magic round to 1/64)
                c1m = epi.tile([128, 512], F32, tag="c1", name="c1")
                te = epi.tile([128, 512], F32, tag="te", name="te")
                to = epi.tile([128, 512], F32, tag="to", name="to")
                nc.scalar.activation(c1m[:P, :Wp], ps[1][:P], ACTF.Copy,
                                     bias=M64)
                nc.vector.tensor_tensor(te[:P, :Wp], c1m[:P, :Wp], ps[0][:P],
                                        OP.add)
                nc.vector.tensor_tensor(to[:P, :Wp], c1m[:P, :Wp], ps[2][:P],
                                        OP.add)
                yi = epi.tile([128, 1024], F32, tag="yi", name="yi")
                wid = 2 * Wp
                # even = (te + MREST) - (-M2); odd = (to + MREST) - M3;
                # the full-magic add rounds y*ss to integer
                nc.vector.scalar_tensor_tensor(
                    yi[:P, 0:Wp], te[:P, :Wp], MREST, ps[2][:P],
                    OP.add, OP.subtract,
                )
                nc.vector.scalar_tensor_tensor(
                    yi[:P, Wp:wid], to[:P, :Wp], MREST, ps[3][:P],
                    OP.add, OP.subtract,
                )
                # clamp+bias chain: clamp in fp32 magic space (DVE 2x),
                # then ScalarE adds (b - MAGF) per partition (de-magic +
                # bias in one activation), then the final 4x fp16 clamp
                w16 = epi.tile([128, 1024], F16, tag="w16", name="w16")
                o16 = epi.tile([128, 1024], F16, tag="o16", name="o16")
                nc.vector.tensor_scalar(
                    yi[:P, :wid], yi[:P, :wid], MAGF + QMAX, MAGF - QMAX,
                    OP.min, OP.max,
                )
                nc.scalar.activation(
                    w16[:P, :wid], yi[:P, :wid], ACTF.Identity,
                    bias=bt[:P, cot : cot + 1],
                )
                nc.vector.tensor_scalar(
                    o16[:P, :wid], w16[:P, :wid], QMAX, -QMAX,
                    OP.min, OP.max,
                )
                # output dram layout is [i, co, eo, r, s]; host de-interleaves
                nps = rows * S
                for part_lo, r0 in yparts:
                    for eo in range(2):
                        nc.sync.dma_start(
                            y_d[i, co0 : co0 + cs,
                                eo * (HW // 2) + r0 * S :
                                eo * (HW // 2) + r0 * S + nps],
                            o16[part_lo : part_lo + cs,
                                eo * Wp : eo * Wp + nps],
                        )

            def emit_group_full(i, p, cot):
                # cout chunks 0/1 (cs=128): one psum bank per comp, free 512
                h, po = divmod(p, 2)
                co0, cs = CHUNKS[cot]
                ps = [psum.tile([128, 512], F32, tag="ps", name=f"ps{_c}")
                      for _c in range(4)]
                for q in (0, 1):
                    for comp in (0, 3, 1, 2):
                        for kh in range(K):
                            nc.tensor.matmul(
                                ps[comp][:cs, :],
                                wslice(comp, kh, q, co0, cs),
                                rhs(q, h, comp, 16 * po + kh, 16),
                                start=(q == 0 and kh == 0),
                                stop=False,
                            )
                # cin remainder: pack comp pairs into the two PE row groups
                # (the 64 cin channels are duplicated on partitions 64:127)
                for kh in range(K):
                    for comp in range(4):
                        lo = 0 if comp % 2 == 0 else 64
                        nc.tensor.matmul(
                            ps[comp][:cs, :],
                            wslice(comp, kh, 2, co0, cs, lo, lo + 64),
                            rhs(2, h, comp, 16 * po + kh, 16, lo, lo + 64),
                            start=False,
                            stop=(kh == 2),
                        )
                emit_outtf_epi(ps, i, cot, co0, cs, 16, [(0, 16 * p)])

            def emit_group_rem(i, pe):
                # column-pack row-pairs pe, pe+1 into the two column halves
                co0, cs = CHUNKS[2]
                h = pe // 2
                poA, poB = pe % 2, (pe + 1) % 2
                ps = [psum.tile([128, 512], F32, tag="ps", name=f"psr{_c}")
                      for _c in range(4)]
                for q in (0, 1):
                    for comp in range(4):
                        for kh in range(K):
                            first = q == 0 and kh == 0
                            w_ = wslice(comp, kh, q, co0, cs)
                            nc.tensor.matmul(
                                ps[comp][0:cs, :], w_,
                                rhs(q, h, comp, 16 * poA + kh, 16),
                                start=first, stop=False,
                                tile_position=(0, 0),
                            )
                            nc.tensor.matmul(
                                ps[comp][64 : 64 + cs, :], w_,
                                rhs(q, h, comp, 16 * poB + kh, 16),
                                start=first, stop=False,
                                tile_position=(0, 64),
                            )
                for comp in range(4):
                    for kh in range(K):
                        last = kh == 2
                        nc.tensor.matmul(
                            ps[comp][0:cs, :],
                            wslice(comp, kh, 2, co0, cs, 0, 64),
                            rhs(2, h, comp, 16 * poA + kh, 16, 0, 64),
                            start=False, stop=last,
                            tile_position=(0, 0),
                        )
                        nc.tensor.matmul(
                            ps[comp][64 : 64 + cs, :],
                            wslice(comp, kh, 2, co0, cs, 64, 128),
                            rhs(2, h, comp, 16 * poB + kh, 16, 64, 128),
                            start=False, stop=last,
                            tile_position=(64, 64),
                        )
                emit_outtf_epi(ps, i, 2, co0, cs, 16,
                               [(0, 16 * pe), (64, 16 * (pe + 1))])

            for i in range(IMGS_PER_CORE):
                for p in range(4):
                    emit_group_full(i, p, 0)
                    # (0,1) staging is a first write (no WAR hazard): spread
                    # its chunks between groups so the DVE bursts stay short
                    if i == 0 and p == 0:
                        emit_x_half(0, 1, only_c=0)
                    emit_group_full(i, p, 1)
                    if i == 0 and p == 0:
                        emit_x_half(0, 1, only_c=1)
                    if p % 2 == 1:
                        emit_group_rem(i, p - 1)
                    if i == 0 and p == 0:
                        emit_x_half(0, 1, only_c=2)
                    # image i+1 staging overwrites tiles read by pairs
                    # 2h, 2h+1 incl. the rem group -> emit only after it
                    if i + 1 < IMGS_PER_CORE and p in (1, 3):
                        for c_ in range(3):
                            emit_x_half(i + 1, p // 2, only_c=c_)

    nc.compile()
    return nc


_BUILD_CACHE = {}


def _get_nc(sx):
    if sx not in _BUILD_CACHE:
        _BUILD_CACHE[sx] = _build(sx)
    return _BUILD_CACHE[sx]


def _run(x, weight, bias, step_x, step_w, step_b, shift_scale, trace=False):
    _install_axon_ntff_hook()
    x = np.ascontiguousarray(np.asarray(x, dtype=np.float32))
    w = np.asarray(weight, dtype=np.float32)
    b = np.ascontiguousarray(np.asarray(bias, dtype=np.float32))
    sx = float(np.asarray(step_x))
    sw = float(np.asarray(step_w))
    sb = float(np.asarray(step_b))
    ss = float(np.asarray(shift_scale))

    nc = _get_nc(sx)

    w_t = prep_weight(w, sw, ss)
    x_sh = x.reshape(N_CORES, IMGS_PER_CORE, CIN, HW)

    # upload b_int8 - MAGF: the epilogue activation's per-partition bias
    # does de-magic and bias add in one pass
    b_i8 = (bias_int8(b, sb, ss, sx, sw) - np.float32(MAGF)).astype(np.float32)
    in_maps = [
        {"x": x_sh[core], "w": w_t, "b": b_i8} for core in range(N_CORES)
    ]
    res = run_bass_kernel_spmd(
        nc, in_maps, core_ids=list(range(N_CORES)), trace=trace
    )
    # device wrote [i, co, eo, r, s] fp16; de-interleave eo into the W axis
    out = np.concatenate(
        [res.results[core]["y"].reshape(IMGS_PER_CORE, COUT, 2, H, S)
         for core in range(N_CORES)],
        axis=0,
    )
    out = np.ascontiguousarray(
        np.transpose(out, (0, 1, 3, 4, 2)).astype(np.float32)
    ).reshape(B, COUT, H, W)
    return out, res


def kernel(x, weight, bias, step_x, step_w, step_b, shift_scale):
    out, _ = _run(x, weight, bias, step_x, step_w, step_b, shift_scale)
    return out


def kernel_profiled(x, weight, bias, step_x, step_w, step_b, shift_scale):
    return _run(x, weight, bias, step_x, step_w, step_b, shift_scale, trace=True)
